# revision 26
# baseline (speedup 1.0000x reference)
"""AUGRU (DIEN attention layer) on 8 Trainium2 NeuronCores via Bass/Tile.

Problem: B=2048, T=200, D=128, H=128 fp32 AUGRU scan with per-row sequence
lengths (zero output + state carry past seq_len).

Strategy v2 (latency-focused redesign; the scan is a 200-step serial
dependency chain, so wall clock ~= sum of per-step critical paths):
  - Pure batch data parallelism, 256 rows per core, feature-major on-chip
    layout [128 features, batch free-dim]; weights stationary, h moving.
  - bf16 datapath: matmuls run 1 cycle/row at ANY moving width (fp32r needs
    >=256), DVE elementwise ops hit the 2x packed mode, HBM traffic halves.
    PSUM accumulation stays fp32; frozen rows (am==0) carry h EXACTLY since
    hn = (h - 0) + 0 in bf16.
  - Rows are sorted by seq_len (descending) per core, so at step t only the
    first m_t columns are still active; every matmul/activation/elementwise
    op shrinks with m_t (~2x less work + shorter per-step latency). Sorting
    is exact, not approximate: inactive rows have am=0 which freezes h, and
    their outputs are zeros that the host fills during unshard.
  - No mask tensor and no GPSIMD ops (the baseline burned ~935ns/op there);
    masked outputs are zero-filled on the host, which also undoes the sort.
  - Outputs are written bf16 in groups of KG steps per DMA.
"""

import os

import numpy as np
import ml_dtypes

import concourse.bacc as bacc
import concourse.mybir as mybir
import concourse.tile as tile
from concourse.bass_utils import run_bass_kernel_spmd

F32 = mybir.dt.float32
BF16 = mybir.dt.bfloat16
AF = mybir.ActivationFunctionType
OP = mybir.AluOpType

B, T, D, H = 2048, 200, 128, 128
NCORES = 8
BL = B // NCORES  # 256 batch rows per core
TB = 25           # timesteps per input DMA block
KG = 4            # timesteps per output DMA group
LA = 1            # x-side matmul emission lookahead (steps)
GRAN = 32         # active-prefix width granularity

LAST_EXEC_TIME_NS = None
_NC_CACHE = {}


def _schedule(m_list):
    """Group steps for output DMA; returns (groups, out_cols).
    groups: list of (t0, n_steps, local_offs, total_cols, col_offset);
    step j of a group lands at col_offset + local_offs[j], width m_list[t0+j]
    (tightly packed so the group DMA reads only written bytes)."""
    groups = []
    off = 0
    t = 0
    while t < len(m_list):
        n = min(KG, len(m_list) - t)
        offs = []
        tot = 0
        for j in range(n):
            offs.append(tot)
            tot += m_list[t + j]
        groups.append((t, n, offs, tot, off))
        off += tot
        t += n
    return groups, off


def _build_kernel(m_list, bg_const, bc_const):
    T_run = len(m_list)
    groups, out_cols = _schedule(m_list)
    nblk = (T_run + TB - 1) // TB

    nc = bacc.Bacc("TRN2", target_bir_lowering=False, debug=False, num_devices=NCORES)

    xT = nc.dram_tensor("xT", [128, T_run * BL], BF16, kind="ExternalInput")
    am = nc.dram_tensor("am", [128, T_run * BL], BF16, kind="ExternalInput")
    wnames = ["wxr", "whr", "whrN", "wxu", "whu", "wxc", "whc"]
    wd = {n: nc.dram_tensor(n, [128, 128], BF16, kind="ExternalInput") for n in wnames}
    if bg_const is None:
        bgr = nc.dram_tensor("bgr", [128, 1], F32, kind="ExternalInput")
        bgu = nc.dram_tensor("bgu", [128, 1], F32, kind="ExternalInput")
    if bc_const is None:
        bcv = nc.dram_tensor("bcv", [128, 1], F32, kind="ExternalInput")
    outT = nc.dram_tensor("outT", [128, out_cols], BF16, kind="ExternalOutput")

    # per-step tile handles filled in during emission
    x_tile = {}   # t -> (tile, col_offset)
    a_tile = {}
    h_loc = {}    # t -> (tile, col_base) of h_t (written at step t)

    with tile.TileContext(nc) as tc:
        with (
            tc.tile_pool(name="w", bufs=1) as wpool,
            tc.tile_pool(name="xb", bufs=2) as xpool,
            tc.tile_pool(name="ab", bufs=2) as apool,
            tc.tile_pool(name="ob", bufs=3) as opool,
            tc.tile_pool(name="s", bufs=3) as spool,
            tc.tile_pool(name="ps", bufs=2, space="PSUM") as ppool,
        ):
            w = {}
            for n in wnames:
                wt = wpool.tile([128, 128], BF16, tag=n, name=f"w_{n}")
                nc.sync.dma_start(wt[:], wd[n].ap())
                w[n] = wt
            btiles = {}
            if bg_const is None:
                for n, dt_ in (("bgr", bgr), ("bgu", bgu)):
                    bt = wpool.tile([128, 1], F32, tag=n, name=f"b_{n}")
                    nc.sync.dma_start(bt[:], dt_.ap())
                    btiles[n] = bt
            if bc_const is None:
                bt = wpool.tile([128, 1], F32, tag="bcv", name="b_bcv")
                nc.sync.dma_start(bt[:], bcv.ap())
                btiles["bcv"] = bt
            bias_r = bg_const if bg_const is not None else btiles["bgr"][:]
            bias_u = bg_const if bg_const is not None else btiles["bgu"][:]
            bias_c = bc_const if bc_const is not None else btiles["bcv"][:]

            mm = nc.tensor.matmul
            tt = nc.vector.tensor_tensor
            stt = nc.vector.scalar_tensor_tensor
            act = nc.scalar.activation

            psum = {}  # t -> dict(pr=, pu=, pc=)

            def emit_block_dma(b):
                lo = b * TB * BL
                hi = min((b + 1) * TB, T_run) * BL
                xb = xpool.tile([128, TB * BL], BF16, tag="xb", name=f"xb_{b}")
                nc.sync.dma_start(xb[:, 0:hi - lo], xT.ap()[:, lo:hi])
                ab = apool.tile([128, TB * BL], BF16, tag="ab", name=f"ab_{b}")
                nc.sync.dma_start(ab[:, 0:hi - lo], am.ap()[:, lo:hi])
                for t in range(b * TB, min((b + 1) * TB, T_run)):
                    off = (t - b * TB) * BL
                    x_tile[t] = (xb, off)
                    a_tile[t] = (ab, off)

            def emit_xside(t):
                m = m_list[t]
                xb, off = x_tile[t]
                xt = xb[:, off:off + m]
                one_shot = (t == 0)  # no h-side accumulation at t=0
                # separate tiles: each lands in its own PSUM bank (a bank can
                # hold only one pending accumulation group)
                pr = None
                if t > 0:  # r-gate is unused at t=0 (r*h = 0)
                    pr = ppool.tile([128, 256], F32, tag="pr", name=f"pr_{t}")
                    mm(pr[:, 0:m], w["wxr"][:], xt, start=True, stop=False)
                pu = ppool.tile([128, 256], F32, tag="pu", name=f"pu_{t}")
                pc = ppool.tile([128, 256], F32, tag="pc", name=f"pc_{t}")
                psum[t] = (pr, pu, pc)
                mm(pu[:, 0:m], w["wxu"][:], xt, start=True, stop=one_shot)
                mm(pc[:, 0:m], w["wxc"][:], xt, start=True, stop=one_shot)

            cur_ob = None  # (tile, group)
            next_xside = [0]

            def pump_xside(upto):
                while (next_xside[0] <= upto and next_xside[0] < T_run
                       and next_xside[0] in x_tile):
                    emit_xside(next_xside[0])
                    next_xside[0] += 1

            v_loc = {}  # t -> v tile (h_t = v_t - wvn_t)
            next_blk = [0]

            def ensure_block_for(ts):
                while next_blk[0] < nblk and next_blk[0] * TB <= ts:
                    emit_block_dma(next_blk[0])
                    next_blk[0] += 1

            for t in range(T_run):
                    m = m_list[t]
                    # make sure x/am for t+1 are loadable so the x-side (and
                    # the whrN pre-accumulate) for t+1 can be emitted below
                    ensure_block_for(min(t + 1, T_run - 1))
                    if t == 0:
                        pump_xside(1)

                    pr, pu, pc = psum[t]
                    ab, aoff = a_tile[t]
                    at = ab[:, aoff:aoff + m]

                    # output staging tile (one per KG-step group)
                    gi = t // KG
                    t0, ngs, loffs, gtot, _goff = groups[gi]
                    if t == t0:
                        obt = opool.tile([128, KG * 256], BF16, tag="ob",
                                         name=f"ob_{gi}")
                        cur_ob = (obt, gi)
                    obt, _ = cur_ob
                    j = t - t0
                    lo = loffs[j]
                    hn = obt[:, lo:lo + m]

                    if t > 0:
                        ht, hb = h_loc[t - 1]
                        h = ht[:, hb:hb + m]
                        # chain head: close the r-gate accumulation.
                        # pr(t) = wxr@x_t [+ whrN@wvn(t-1)] + whr@{v(t-1)|h},
                        # the x/wvn parts were emitted during earlier steps.
                        if t >= 2:
                            vm = v_loc[t - 1]
                            mm(pr[:, 0:m], w["whr"][:], vm[:, 0:m],
                               start=False, stop=True)
                        else:
                            mm(pr[:, 0:m], w["whr"][:], h,
                               start=False, stop=True)
                        mm(pu[:, 0:m], w["whu"][:], h, start=False, stop=True)
                    # x-side matmuls for t+1 fill the PE while the chain
                    # waits on sigmoid/rh
                    pump_xside(t + 1)

                    gu = spool.tile([128, 256], BF16, tag="gu", name=f"gu_{t}")
                    if t > 0:
                        # r = sigmoid(pr + bg), on the chain
                        gr = spool.tile([128, 256], BF16, tag="gr",
                                        name=f"gr_{t}")
                        act(gr[:, 0:m], pr[:, 0:m], AF.Sigmoid, bias=bias_r)
                        act(gu[:, 0:m], pu[:, 0:m], AF.Sigmoid, bias=bias_u)
                        rh = spool.tile([128, 256], BF16, tag="rh",
                                        name=f"rh_{t}")
                        tt(rh[:, 0:m], gr[:, 0:m], h, OP.mult)
                        mm(pc[:, 0:m], w["whc"][:], rh[:, 0:m],
                           start=False, stop=True)
                        up = spool.tile([128, 256], BF16, tag="up",
                                        name=f"up_{t}")
                        tt(up[:, 0:m], gu[:, 0:m], at, OP.mult)
                        # wvn = (up-1)*h = -(1-u')h, off the chain
                        wvn = spool.tile([128, 256], BF16, tag="wvn",
                                         name=f"wvn_{t}")
                        stt(wvn[:, 0:m], up[:, 0:m], 1.0, h,
                            OP.subtract, OP.mult)
                        # pre-accumulate whrN@wvn into pr(t+1) (h_t = v - wvn)
                        if t + 1 < T_run:
                            m2 = m_list[t + 1]
                            mm(psum[t + 1][0][:, 0:m2], w["whrN"][:],
                               wvn[:, 0:m2], start=False, stop=False)
                    else:
                        act(gu[:, 0:m], pu[:, 0:m], AF.Sigmoid, bias=bias_u)
                        up = spool.tile([128, 256], BF16, tag="up",
                                        name=f"up_{t}")
                        tt(up[:, 0:m], gu[:, 0:m], at, OP.mult)
                    cc = spool.tile([128, 256], BF16, tag="cc", name=f"cc_{t}")
                    act(cc[:, 0:m], pc[:, 0:m], AF.Tanh, bias=bias_c)
                    if t > 0:
                        v = spool.tile([128, 256], BF16, tag="v", name=f"v_{t}")
                        tt(v[:, 0:m], up[:, 0:m], cc[:, 0:m], OP.mult)
                        # h_t = v - wvn, computed off the chain (consumers of
                        # h_t other than pr(t+1) run later)
                        tt(hn, v[:, 0:m], wvn[:, 0:m], OP.subtract)
                        v_loc[t] = v
                    else:
                        tt(hn, up[:, 0:m], cc[:, 0:m], OP.mult)
                    h_loc[t] = (obt, lo)

                    if t == t0 + ngs - 1:
                        goff = groups[gi][4]
                        nc.sync.dma_start(
                            outT.ap()[:, goff:goff + gtot],
                            obt[:, 0:gtot])
    nc.compile()
    return nc, groups, out_cols


def _prep_inputs(inputs, att_scores, seq_len, Wg, bg, Wc, bc):
    x = np.asarray(inputs, dtype=np.float32)
    att = np.asarray(att_scores, dtype=np.float32)
    sl = np.asarray(seq_len, dtype=np.int64)
    Wg = np.asarray(Wg, dtype=np.float32)
    bg = np.asarray(bg, dtype=np.float32)
    Wc = np.asarray(Wc, dtype=np.float32)
    bc = np.asarray(bc, dtype=np.float32)
    BF = ml_dtypes.bfloat16

    # global sort by seq_len desc, deal round-robin to cores
    order = np.argsort(-sl, kind="stable")
    perms = [order[c::NCORES] for c in range(NCORES)]  # each len BL, desc

    T_run = int(sl.max())  # steps t >= T_run produce nothing
    # per-core active counts; m_t = padded max across cores
    k_core = np.zeros((NCORES, T_run), np.int64)
    for c in range(NCORES):
        slc = sl[perms[c]]  # descending
        for t in range(T_run):
            k_core[c, t] = np.searchsorted(-slc, -(t + 1), side="right")
    m_list = []
    for t in range(T_run):
        k = int(k_core[:, t].max())
        m = min(BL, ((k + GRAN - 1) // GRAN) * GRAN)
        m_list.append(max(m, GRAN))
    # enforce non-increasing (k is non-increasing already; padding keeps it)
    for t in range(1, T_run):
        m_list[t] = min(m_list[t], m_list[t - 1])

    bg_const = float(bg.flat[0]) if np.all(bg == bg.flat[0]) else None
    bc_const = float(bc.flat[0]) if np.all(bc == bc.flat[0]) else None

    m16 = (np.arange(T, dtype=np.int64)[None, :] < sl[:, None])
    amf = att * m16.astype(np.float32)

    wmats = {
        "wxr": Wg[0:128, 0:128], "whr": Wg[128:256, 0:128],
        "whrN": -Wg[128:256, 0:128],
        "wxu": Wg[0:128, 128:256], "whu": Wg[128:256, 128:256],
        "wxc": Wc[0:128, :], "whc": Wc[128:256, :],
    }
    wmats = {k: np.ascontiguousarray(v).astype(BF) for k, v in wmats.items()}

    in_maps = []
    for c in range(NCORES):
        p = perms[c]
        xk = np.ascontiguousarray(
            x[p, :T_run].transpose(2, 1, 0)).astype(BF)      # [D, T_run, BL]
        amk = np.ascontiguousarray(
            np.broadcast_to(amf[p, :T_run].T[None, :, :].astype(BF),
                            (128, T_run, BL)))
        im = {
            "xT": xk.reshape(128, T_run * BL),
            "am": amk.reshape(128, T_run * BL),
            **wmats,
        }
        if bg_const is None:
            im["bgr"] = np.ascontiguousarray(bg[0:128, None])
            im["bgu"] = np.ascontiguousarray(bg[128:256, None])
        if bc_const is None:
            im["bcv"] = np.ascontiguousarray(bc[:, None])
        in_maps.append(im)
    return in_maps, perms, k_core, m_list, bg_const, bc_const


def kernel(inputs, att_scores, seq_len, Wg, bg, Wc, bc):
    global LAST_EXEC_TIME_NS
    in_maps, perms, k_core, m_list, bg_const, bc_const = _prep_inputs(
        inputs, att_scores, seq_len, Wg, bg, Wc, bc)

    key = (tuple(m_list), bg_const, bc_const)
    if key not in _NC_CACHE:
        _NC_CACHE[key] = _build_kernel(m_list, bg_const, bc_const)
    nc, groups, out_cols = _NC_CACHE[key]

    trace = bool(int(os.environ.get("AUGRU_TRACE", "0")))
    kwargs = {}
    if trace:
        kwargs["trace"] = True
        tmpdir = os.environ.get("AUGRU_TRACE_DIR")
        if tmpdir:
            os.makedirs(tmpdir, exist_ok=True)
            kwargs["tmpdir"] = tmpdir
    try:
        res = run_bass_kernel_spmd(nc, in_maps, list(range(NCORES)), **kwargs)
    except Exception:
        if not kwargs:
            raise
        res = run_bass_kernel_spmd(nc, in_maps, list(range(NCORES)))
    LAST_EXEC_TIME_NS = res.exec_time_ns

    out = np.zeros((B, T, H), np.float32)
    for c in range(NCORES):
        o = res.results[c]["outT"]  # [128, out_cols] bf16
        p = perms[c]
        for (t0, ngs, loffs, gtot, goff) in groups:
            for j in range(ngs):
                t = t0 + j
                k = int(k_core[c, t])
                if k == 0:
                    continue
                sub = np.asarray(
                    o[:, goff + loffs[j]: goff + loffs[j] + k],
                    dtype=np.float32)
                out[p[:k], t, :] = sub.T
    return out


# revision 31
# speedup vs baseline: 1.0956x; 1.0956x over previous
"""AUGRU (DIEN attention layer) on 8 Trainium2 NeuronCores via Bass/Tile.

Problem: B=2048, T=200, D=128, H=128 fp32 AUGRU scan with per-row sequence
lengths (zero output + state carry past seq_len).

Strategy v2 (latency-focused redesign; the scan is a 200-step serial
dependency chain, so wall clock ~= sum of per-step critical paths):
  - Pure batch data parallelism, 256 rows per core, feature-major on-chip
    layout [128 features, batch free-dim]; weights stationary, h moving.
  - bf16 datapath: matmuls run 1 cycle/row at ANY moving width (fp32r needs
    >=256), DVE elementwise ops hit the 2x packed mode, HBM traffic halves.
    PSUM accumulation stays fp32; frozen rows (am==0) carry h EXACTLY since
    hn = (h - 0) + 0 in bf16.
  - Rows are sorted by seq_len (descending) per core, so at step t only the
    first m_t columns are still active; every matmul/activation/elementwise
    op shrinks with m_t (~2x less work + shorter per-step latency). Sorting
    is exact, not approximate: inactive rows have am=0 which freezes h, and
    their outputs are zeros that the host fills during unshard.
  - No mask tensor and no GPSIMD ops (the baseline burned ~935ns/op there);
    masked outputs are zero-filled on the host, which also undoes the sort.
  - Outputs are written bf16 in groups of KG steps per DMA.
"""

import os

import numpy as np
import ml_dtypes

import concourse.bacc as bacc
import concourse.mybir as mybir
import concourse.tile as tile
from concourse.bass_utils import run_bass_kernel_spmd

F32 = mybir.dt.float32
BF16 = mybir.dt.bfloat16
AF = mybir.ActivationFunctionType
OP = mybir.AluOpType

B, T, D, H = 2048, 200, 128, 128
NCORES = 8
BL = B // NCORES  # 256 batch rows per core
TB = 25           # timesteps per input DMA block
KG = 4            # timesteps per output DMA group
LA = 1            # x-side matmul emission lookahead (steps)
GRAN = 16         # active-prefix width granularity

LAST_EXEC_TIME_NS = None
_NC_CACHE = {}


def _schedule(m_list):
    """Group steps for output DMA; returns (groups, out_cols).
    groups: list of (t0, n_steps, local_offs, total_cols, col_offset);
    step j of a group lands at col_offset + local_offs[j], width m_list[t0+j]
    (tightly packed so the group DMA reads only written bytes)."""
    groups = []
    off = 0
    t = 0
    while t < len(m_list):
        n = min(KG, len(m_list) - t)
        offs = []
        tot = 0
        for j in range(n):
            offs.append(tot)
            tot += m_list[t + j]
        groups.append((t, n, offs, tot, off))
        off += tot
        t += n
    return groups, off


def _build_kernel(m_list, bg_const, bc_const):
    T_run = len(m_list)
    groups, out_cols = _schedule(m_list)
    nblk = (T_run + TB - 1) // TB

    nc = bacc.Bacc("TRN2", target_bir_lowering=False, debug=False, num_devices=NCORES)

    xT = nc.dram_tensor("xT", [128, T_run * BL], BF16, kind="ExternalInput")
    am = nc.dram_tensor("am", [128, T_run * BL], BF16, kind="ExternalInput")
    wnames = ["wxr", "whr", "wxu", "whu", "wxc", "whc"]
    wd = {n: nc.dram_tensor(n, [128, 128], BF16, kind="ExternalInput") for n in wnames}
    if bg_const is None:
        bgr = nc.dram_tensor("bgr", [128, 1], F32, kind="ExternalInput")
        bgu = nc.dram_tensor("bgu", [128, 1], F32, kind="ExternalInput")
    if bc_const is None:
        bcv = nc.dram_tensor("bcv", [128, 1], F32, kind="ExternalInput")
    outT = nc.dram_tensor("outT", [128, out_cols], BF16, kind="ExternalOutput")

    # per-step tile handles filled in during emission
    x_tile = {}   # t -> (tile, col_offset)
    a_tile = {}
    h_loc = {}    # t -> (tile, col_base) of h_t (written at step t)

    with tile.TileContext(nc) as tc:
        with (
            tc.tile_pool(name="w", bufs=1) as wpool,
            tc.tile_pool(name="xb", bufs=2) as xpool,
            tc.tile_pool(name="ab", bufs=2) as apool,
            tc.tile_pool(name="ob", bufs=3) as opool,
            tc.tile_pool(name="s", bufs=3) as spool,
            tc.tile_pool(name="ps", bufs=2, space="PSUM") as ppool,
        ):
            w = {}
            for n in wnames:
                wt = wpool.tile([128, 128], BF16, tag=n, name=f"w_{n}")
                nc.sync.dma_start(wt[:], wd[n].ap())
                w[n] = wt
            btiles = {}
            if bg_const is None:
                for n, dt_ in (("bgr", bgr), ("bgu", bgu)):
                    bt = wpool.tile([128, 1], F32, tag=n, name=f"b_{n}")
                    nc.sync.dma_start(bt[:], dt_.ap())
                    btiles[n] = bt
            if bc_const is None:
                bt = wpool.tile([128, 1], F32, tag="bcv", name="b_bcv")
                nc.sync.dma_start(bt[:], bcv.ap())
                btiles["bcv"] = bt
            bias_r = bg_const if bg_const is not None else btiles["bgr"][:]
            bias_u = bg_const if bg_const is not None else btiles["bgu"][:]
            bias_c = bc_const if bc_const is not None else btiles["bcv"][:]

            mm = nc.tensor.matmul
            tt = nc.vector.tensor_tensor
            stt = nc.vector.scalar_tensor_tensor
            act = nc.scalar.activation

            psum = {}  # t -> dict(pr=, pu=, pc=)

            def emit_block_dma(b):
                lo = b * TB * BL
                hi = min((b + 1) * TB, T_run) * BL
                xb = xpool.tile([128, TB * BL], BF16, tag="xb", name=f"xb_{b}")
                nc.sync.dma_start(xb[:, 0:hi - lo], xT.ap()[:, lo:hi])
                ab = apool.tile([128, TB * BL], BF16, tag="ab", name=f"ab_{b}")
                nc.sync.dma_start(ab[:, 0:hi - lo], am.ap()[:, lo:hi])
                for t in range(b * TB, min((b + 1) * TB, T_run)):
                    off = (t - b * TB) * BL
                    x_tile[t] = (xb, off)
                    a_tile[t] = (ab, off)

            def emit_xside(t):
                m = m_list[t]
                xb, off = x_tile[t]
                xt = xb[:, off:off + m]
                one_shot = (t == 0)  # no h-side accumulation at t=0
                # separate tiles: each lands in its own PSUM bank (a bank can
                # hold only one pending accumulation group)
                pr = None
                if t > 0:  # r-gate is unused at t=0 (r*h = 0)
                    pr = ppool.tile([128, 256], F32, tag="pr", name=f"pr_{t}")
                    mm(pr[:, 0:m], w["wxr"][:], xt, start=True, stop=False)
                pu = ppool.tile([128, 256], F32, tag="pu", name=f"pu_{t}")
                pc = ppool.tile([128, 256], F32, tag="pc", name=f"pc_{t}")
                psum[t] = (pr, pu, pc)
                mm(pu[:, 0:m], w["wxu"][:], xt, start=True, stop=one_shot)
                mm(pc[:, 0:m], w["wxc"][:], xt, start=True, stop=one_shot)

            cur_ob = None  # (tile, group)
            next_xside = [0]

            def pump_xside(upto):
                while (next_xside[0] <= upto and next_xside[0] < T_run
                       and next_xside[0] in x_tile):
                    emit_xside(next_xside[0])
                    next_xside[0] += 1

            next_blk = [0]

            def ensure_block_for(ts):
                while next_blk[0] < nblk and next_blk[0] * TB <= ts:
                    emit_block_dma(next_blk[0])
                    next_blk[0] += 1

            for t in range(T_run):
                    m = m_list[t]
                    # make sure x/am for t+1 are loadable so the x-side (and
                    # the whrN pre-accumulate) for t+1 can be emitted below
                    ensure_block_for(min(t + 1, T_run - 1))
                    if t == 0:
                        pump_xside(1)

                    pr, pu, pc = psum[t]
                    ab, aoff = a_tile[t]
                    at = ab[:, aoff:aoff + m]

                    # output staging tile (one per KG-step group)
                    gi = t // KG
                    t0, ngs, loffs, gtot, _goff = groups[gi]
                    if t == t0:
                        obt = opool.tile([128, KG * 256], BF16, tag="ob",
                                         name=f"ob_{gi}")
                        cur_ob = (obt, gi)
                    obt, _ = cur_ob
                    j = t - t0
                    lo = loffs[j]
                    hn = obt[:, lo:lo + m]

                    if t > 0:
                        ht, hb = h_loc[t - 1]
                        h = ht[:, hb:hb + m]
                        # chain head; mm_r first so sigmoid(r) can start
                        # while mm_u still runs
                        mm(pr[:, 0:m], w["whr"][:], h, start=False, stop=True)
                        mm(pu[:, 0:m], w["whu"][:], h, start=False, stop=True)
                    # x-side matmuls for t+1 sit behind the chain mms in the
                    # PE queue: they bypass-fill the sigmoid/rh window but
                    # never delay a ready chain matmul
                    pump_xside(t + 1)

                    gu = spool.tile([128, 256], BF16, tag="gu", name=f"gu_{t}")
                    if t > 0:
                        # r = sigmoid(pr + bg), on the chain
                        gr = spool.tile([128, 256], BF16, tag="gr",
                                        name=f"gr_{t}")
                        act(gr[:, 0:m], pr[:, 0:m], AF.Sigmoid, bias=bias_r)
                        act(gu[:, 0:m], pu[:, 0:m], AF.Sigmoid, bias=bias_u)
                        rh = spool.tile([128, 256], BF16, tag="rh",
                                        name=f"rh_{t}")
                        tt(rh[:, 0:m], gr[:, 0:m], h, OP.mult)
                        mm(pc[:, 0:m], w["whc"][:], rh[:, 0:m],
                           start=False, stop=True)
                        up = spool.tile([128, 256], BF16, tag="up",
                                        name=f"up_{t}")
                        tt(up[:, 0:m], gu[:, 0:m], at, OP.mult)
                        # wvn = (up-1)*h = -(1-u')h, off the chain
                        wvn = spool.tile([128, 256], BF16, tag="wvn",
                                         name=f"wvn_{t}")
                        stt(wvn[:, 0:m], up[:, 0:m], 1.0, h,
                            OP.subtract, OP.mult)
                    else:
                        act(gu[:, 0:m], pu[:, 0:m], AF.Sigmoid, bias=bias_u)
                        up = spool.tile([128, 256], BF16, tag="up",
                                        name=f"up_{t}")
                        tt(up[:, 0:m], gu[:, 0:m], at, OP.mult)
                    cc = spool.tile([128, 256], BF16, tag="cc", name=f"cc_{t}")
                    act(cc[:, 0:m], pc[:, 0:m], AF.Tanh, bias=bias_c)
                    if t > 0:
                        v = spool.tile([128, 256], BF16, tag="v", name=f"v_{t}")
                        tt(v[:, 0:m], up[:, 0:m], cc[:, 0:m], OP.mult)
                        tt(hn, v[:, 0:m], wvn[:, 0:m], OP.subtract)
                    else:
                        tt(hn, up[:, 0:m], cc[:, 0:m], OP.mult)
                    h_loc[t] = (obt, lo)

                    if t == t0 + ngs - 1:
                        goff = groups[gi][4]
                        nc.sync.dma_start(
                            outT.ap()[:, goff:goff + gtot],
                            obt[:, 0:gtot])
    nc.compile()
    return nc, groups, out_cols


def _prep_inputs(inputs, att_scores, seq_len, Wg, bg, Wc, bc):
    x = np.asarray(inputs, dtype=np.float32)
    att = np.asarray(att_scores, dtype=np.float32)
    sl = np.asarray(seq_len, dtype=np.int64)
    Wg = np.asarray(Wg, dtype=np.float32)
    bg = np.asarray(bg, dtype=np.float32)
    Wc = np.asarray(Wc, dtype=np.float32)
    bc = np.asarray(bc, dtype=np.float32)
    BF = ml_dtypes.bfloat16

    # global sort by seq_len desc, deal round-robin to cores
    order = np.argsort(-sl, kind="stable")
    perms = [order[c::NCORES] for c in range(NCORES)]  # each len BL, desc

    T_run = int(sl.max())  # steps t >= T_run produce nothing
    # per-core active counts; m_t = padded max across cores
    k_core = np.zeros((NCORES, T_run), np.int64)
    for c in range(NCORES):
        slc = sl[perms[c]]  # descending
        for t in range(T_run):
            k_core[c, t] = np.searchsorted(-slc, -(t + 1), side="right")
    m_list = []
    for t in range(T_run):
        k = int(k_core[:, t].max())
        m = min(BL, ((k + GRAN - 1) // GRAN) * GRAN)
        m_list.append(max(m, GRAN))
    # enforce non-increasing (k is non-increasing already; padding keeps it)
    for t in range(1, T_run):
        m_list[t] = min(m_list[t], m_list[t - 1])

    bg_const = float(bg.flat[0]) if np.all(bg == bg.flat[0]) else None
    bc_const = float(bc.flat[0]) if np.all(bc == bc.flat[0]) else None

    m16 = (np.arange(T, dtype=np.int64)[None, :] < sl[:, None])
    amf = att * m16.astype(np.float32)

    wmats = {
        "wxr": Wg[0:128, 0:128], "whr": Wg[128:256, 0:128],
        "wxu": Wg[0:128, 128:256], "whu": Wg[128:256, 128:256],
        "wxc": Wc[0:128, :], "whc": Wc[128:256, :],
    }
    wmats = {k: np.ascontiguousarray(v).astype(BF) for k, v in wmats.items()}

    in_maps = []
    for c in range(NCORES):
        p = perms[c]
        xk = np.ascontiguousarray(
            x[p, :T_run].transpose(2, 1, 0)).astype(BF)      # [D, T_run, BL]
        amk = np.ascontiguousarray(
            np.broadcast_to(amf[p, :T_run].T[None, :, :].astype(BF),
                            (128, T_run, BL)))
        im = {
            "xT": xk.reshape(128, T_run * BL),
            "am": amk.reshape(128, T_run * BL),
            **wmats,
        }
        if bg_const is None:
            im["bgr"] = np.ascontiguousarray(bg[0:128, None])
            im["bgu"] = np.ascontiguousarray(bg[128:256, None])
        if bc_const is None:
            im["bcv"] = np.ascontiguousarray(bc[:, None])
        in_maps.append(im)
    return in_maps, perms, k_core, m_list, bg_const, bc_const


def kernel(inputs, att_scores, seq_len, Wg, bg, Wc, bc):
    global LAST_EXEC_TIME_NS
    in_maps, perms, k_core, m_list, bg_const, bc_const = _prep_inputs(
        inputs, att_scores, seq_len, Wg, bg, Wc, bc)

    key = (tuple(m_list), bg_const, bc_const)
    if key not in _NC_CACHE:
        _NC_CACHE[key] = _build_kernel(m_list, bg_const, bc_const)
    nc, groups, out_cols = _NC_CACHE[key]

    trace = bool(int(os.environ.get("AUGRU_TRACE", "0")))
    kwargs = {}
    if trace:
        kwargs["trace"] = True
        tmpdir = os.environ.get("AUGRU_TRACE_DIR")
        if tmpdir:
            os.makedirs(tmpdir, exist_ok=True)
            kwargs["tmpdir"] = tmpdir
    try:
        res = run_bass_kernel_spmd(nc, in_maps, list(range(NCORES)), **kwargs)
    except Exception:
        if not kwargs:
            raise
        res = run_bass_kernel_spmd(nc, in_maps, list(range(NCORES)))
    LAST_EXEC_TIME_NS = res.exec_time_ns

    out = np.zeros((B, T, H), np.float32)
    for c in range(NCORES):
        o = res.results[c]["outT"]  # [128, out_cols] bf16
        p = perms[c]
        for (t0, ngs, loffs, gtot, goff) in groups:
            for j in range(ngs):
                t = t0 + j
                k = int(k_core[c, t])
                if k == 0:
                    continue
                sub = np.asarray(
                    o[:, goff + loffs[j]: goff + loffs[j] + k],
                    dtype=np.float32)
                out[p[:k], t, :] = sub.T
    return out
